# revision 14
# baseline (speedup 1.0000x reference)
"""Trainium2 Bass kernel for nn_AuxCMP_61907658604772 (retrieval_knn).

Reference semantics (only the last time step of d/m matters):
    data = d[:, -1].reshape(B, C, S2)            # [64, 64, 1024] f32
    mask = m[:, -1].reshape(B, C, S2)            # [64, 64, 1024] i32 (0/1)
    cell_empty = (mask.sum(axis=(0, 1)) == 0)    # [1024] per-cell predicate
    gathered = data[:, :, poi_index]             # gather along cell dim
    out = (data + where(cell_empty, gathered, 0)).reshape(B, C, 32, 32)

Sharding: by CELLS — core k owns cells [128k, 128(k+1)) x all 4096 (b, c)
rows, in cell-major ("transposed") layout, so the empty predicate is a
core-local reduce over the cell's packed mask row and there is no
collective (an AllReduce variant measured 66us of peer-wait).

v6 design (f32 baseline 29.2us; bf16 variants v2/v4 25.4/25.7us):
  * bf16 end-to-end (harness gate is rel_err < 2e-2, bf16 costs ~4e-3):
    halves every transfer.  2.56MB/core total HBM traffic.
  * The gather ACCUMULATES into the data tiles (indirect_dma_start with
    compute_op=add); non-empty cells' indices are pushed out of bounds so
    their descriptors are skipped.  There is NO element-wise combine at
    all (a fused DVE mult+add runs ~1.04ns/col = 4.3us serial for the
    full 4096 columns — measured, no 16-bit speedup) and no memzero
    (skipped rows keep dc = data, which IS the answer for those cells).
  * mask + idx descriptors are issued ahead of the 1MB of data-slice
    descriptors: DMA queues are FIFO, so anything issued later completes
    after everything already enqueued — the predicate inputs must go
    first (this cost v2 ~3us).
  * Stores are split across the two HWDGE engines (SP + Activation) so
    the two store issues don't serialize on one sequencer.
"""

import numpy as np
import ml_dtypes

from concourse import bacc, bass, mybir, tile
from concourse.bass_utils import run_bass_kernel_spmd

N_CORES = 8
B, T, C, S2 = 64, 12, 64, 1024
SIDE = 32
ALL_ROWS = B * C                # 4096 (b, c) rows per cell
PACKED = ALL_ROWS // 8          # 512 packed mask bytes per cell
P = 128                         # SBUF partitions = cells per core
NCH = 2                         # chunks over the 4096 rows (= gather splits)
CHW = ALL_ROWS // NCH           # rows per chunk
OOB = 65536.0                   # shift pushing non-empty cells' gathers OOB

_CACHE = {}


def _build_program():
    nc = bacc.Bacc(
        "TRN2",
        target_bir_lowering=False,
        debug=False,
        num_devices=N_CORES,
    )
    # data_full (bf16, transposed, replicated) viewed as chunk-rows: cell
    # c's columns [CHW*h, CHW*(h+1)) live in row NCH*c + h.
    data_g = nc.dram_tensor(
        "data_g", [NCH * S2, CHW], mybir.dt.bfloat16, kind="ExternalInput"
    ).ap()
    data_slice = nc.dram_tensor(
        "data_slice", [P, ALL_ROWS], mybir.dt.bfloat16, kind="ExternalInput"
    ).ap()
    maskp = nc.dram_tensor(
        "maskp", [P, PACKED], mybir.dt.uint8, kind="ExternalInput"
    ).ap()
    # idx[p, h] = NCH*poi[cell] + h
    idx4 = nc.dram_tensor("idx4", [P, NCH], mybir.dt.int32, kind="ExternalInput").ap()
    out_t = nc.dram_tensor(
        "out_t", [P, ALL_ROWS], mybir.dt.bfloat16, kind="ExternalOutput"
    ).ap()

    with tile.TileContext(nc) as tc:
        with tc.tile_pool(name="sbuf", bufs=1) as pool:
            # mask + idx descriptors first into the (FIFO) DMA engines.
            mp = pool.tile([P, PACKED], mybir.dt.uint8, tag="mask")
            nc.scalar.dma_start(out=mp[:], in_=maskp[:])
            idx_sb = pool.tile([P, NCH], mybir.dt.int32, tag="idx")
            nc.sync.dma_start(out=idx_sb[:], in_=idx4[:])

            # ---- data loads, chunked over rows ----
            dcs = []
            for c in range(NCH):
                dc = pool.tile([P, CHW], mybir.dt.bfloat16, tag=f"d{c}")
                nc.sync.dma_start(
                    out=dc[:], in_=data_slice[:, c * CHW : (c + 1) * CHW]
                )
                dcs.append(dc)

            # ---- predicate -> idx_eff on DVE (runs under the dc loads).
            # idx_f copy first: it depends only on the idx load.
            idx_f = pool.tile([P, NCH], mybir.dt.float32, tag="idxf")
            nc.vector.tensor_copy(out=idx_f[:], in_=idx_sb[:])
            mmax = pool.tile([P, 1], mybir.dt.float32, tag="mmax")
            nc.vector.tensor_reduce(
                out=mmax[:],
                in_=mp[:],
                axis=mybir.AxisListType.X,
                op=mybir.AluOpType.max,
            )
            # shift = min(mmax, 1) * OOB: 0 for empty cells, OOB otherwise
            shift = pool.tile([P, 1], mybir.dt.float32, tag="shift")
            nc.vector.tensor_scalar(
                out=shift[:],
                in0=mmax[:],
                scalar1=1.0,
                scalar2=OOB,
                op0=mybir.AluOpType.min,
                op1=mybir.AluOpType.mult,
            )
            nc.vector.tensor_scalar(
                out=idx_f[:],
                in0=idx_f[:],
                scalar1=shift[:, 0:1],
                scalar2=None,
                op0=mybir.AluOpType.add,
            )
            idx_eff = pool.tile([P, NCH], mybir.dt.int32, tag="idxe")
            nc.vector.tensor_copy(out=idx_eff[:], in_=idx_f[:])

            # ---- gather-accumulate: dc[p, :] += data_g[idx_eff[p, h], :]
            # for empty cells; OOB descriptors are skipped so non-empty
            # cells keep dc = data (exactly the reference semantics).
            store_eng = [nc.scalar, nc.sync]
            for h in range(NCH):
                nc.gpsimd.indirect_dma_start(
                    out=dcs[h][:],
                    out_offset=None,
                    in_=data_g[:, :],
                    in_offset=bass.IndirectOffsetOnAxis(
                        ap=idx_eff[:, h : h + 1], axis=0
                    ),
                    bounds_check=NCH * S2 - 1,
                    oob_is_err=False,
                    compute_op=mybir.AluOpType.add,
                )
                store_eng[h % 2].dma_start(
                    out=out_t[:, h * CHW : (h + 1) * CHW], in_=dcs[h][:]
                )

    nc.compile()
    return nc


def _get_program():
    if "nc" not in _CACHE:
        _CACHE["nc"] = _build_program()
    return _CACHE["nc"]


def _marshal(d, m, poi_index):
    d = np.asarray(d)
    m = np.asarray(m)
    poi_index = np.asarray(poi_index)

    # Full transposed views: [1024 cells, 4096 rows], bf16
    data_full = np.ascontiguousarray(d[:, -1].reshape(ALL_ROWS, S2).T).astype(
        ml_dtypes.bfloat16
    )
    maskp_full = np.packbits(
        m[:, -1].reshape(ALL_ROWS, S2).T != 0, axis=1
    )  # [1024, 512] u8

    poi = poi_index.astype(np.int32)

    data_g = data_full.reshape(NCH * S2, CHW)  # view, no copy

    in_maps = []
    for k in range(N_CORES):
        cells = slice(k * P, (k + 1) * P)
        idx4 = np.ascontiguousarray(
            NCH * poi[cells, None] + np.arange(NCH, dtype=np.int32)[None, :]
        )  # [128, NCH]
        in_maps.append(
            {
                "data_g": data_g,
                "data_slice": data_full[cells],
                "maskp": maskp_full[cells],
                "idx4": idx4,
            }
        )
    return in_maps


def _unmarshal(results):
    # results[k]["out_t"] is [128 cells, 4096 rows] bf16; rows = b*64 + c.
    out = np.concatenate(
        [np.asarray(r["out_t"]) for r in results], axis=0
    )  # [1024, 4096]
    out = out.astype(np.float32).T.reshape(B, C, S2)  # [64, 64, 1024]
    return np.ascontiguousarray(out.reshape(B, C, SIDE, SIDE))


def run(d, m, poi_index, side, trace=False):
    """Run the Bass kernel; returns (output, BassKernelResults)."""
    nc = _get_program()
    in_maps = _marshal(d, m, poi_index)
    res = run_bass_kernel_spmd(
        nc, in_maps, list(range(N_CORES)), trace=trace
    )
    return _unmarshal(res.results), res


def kernel(d, m, poi_index, side):
    out, _ = run(d, m, poi_index, side)
    return out


# revision 20
# speedup vs baseline: 1.0311x; 1.0311x over previous
"""Trainium2 Bass kernel for nn_AuxCMP_61907658604772 (retrieval_knn).

Reference semantics (only the last time step of d/m matters):
    data = d[:, -1].reshape(B, C, S2)            # [64, 64, 1024] f32
    mask = m[:, -1].reshape(B, C, S2)            # [64, 64, 1024] i32 (0/1)
    cell_empty = (mask.sum(axis=(0, 1)) == 0)    # [1024] per-cell predicate
    gathered = data[:, :, poi_index]             # gather along cell dim
    out = (data + where(cell_empty, gathered, 0)).reshape(B, C, 32, 32)

Sharding: by CELLS — core k owns cells [128k, 128(k+1)) x all 4096 (b, c)
rows, in cell-major ("transposed") layout, so the empty predicate is a
core-local reduce over the cell's packed mask row and there is no
collective (an AllReduce variant measured 66us of peer-wait).

v7 design (f32 baseline 29.2us; bf16 single-method variants 25.4-26.9us):
bf16 end-to-end (harness gate is rel_err < 2e-2, bf16 costs ~4e-3), and
the row range is split between TWO independent pipelines whose serial
chains overlap in time:

  Half A (cols [0, AW)) — gather-ACCUMULATE: indirect DMA with
    compute_op=add lands gathered rows directly into the data tile;
    non-empty cells' indices are pushed out of bounds so their
    descriptors are skipped (dc stays = data).  No element-wise combine,
    but the launch must wait for the data tile + the mask predicate.
  Half B (cols [AW, 4096)) — SPECULATIVE gather: every cell pulls its
    poi row the moment the idx load lands (no predicate, no data-tile
    dependency), and one fused DVE scalar_tensor_tensor applies
    dcB = gB*empty + dcB while half A's gather chain is in flight.

Single-method variants bottom out ~25.5us because each method's serial
chain (dc land -> launch 1.8us -> gather flow -> sem 0.9 -> store) or
(gather land -> 2.3us/2KB-col DVE combine -> store) stacks onto the
~11us fixed floor + ~8us of bus time; splitting runs the two chains
concurrently.

Per-core HBM traffic: 1MB slice + ~0.75MB gather + 64KB mask + 1MB out.
"""

import numpy as np
import ml_dtypes

from concourse import bacc, bass, mybir, tile
from concourse.bass_utils import run_bass_kernel_spmd

N_CORES = 8
B, T, C, S2 = 64, 12, 64, 1024
SIDE = 32
ALL_ROWS = B * C                # 4096 (b, c) rows per cell
PACKED = ALL_ROWS // 8          # 512 packed mask bytes per cell
P = 128                         # SBUF partitions = cells per core
HALF = ALL_ROWS // 2            # data_g half-row width (host layout)
AW = 2048                       # cols on the accumulate path (A = half 0)
BW = ALL_ROWS - AW              # cols on the speculative+combine path
OOB = 65536.0                   # shift pushing non-empty cells' gathers OOB

_CACHE = {}


def _build_program():
    nc = bacc.Bacc(
        "TRN2",
        target_bir_lowering=False,
        debug=False,
        num_devices=N_CORES,
    )
    # data_full (bf16, transposed, replicated), split by columns into the
    # two pipelines' source arrays (indirect DMA needs offset-0 sources).
    data_ga = nc.dram_tensor(
        "data_ga", [S2, AW], mybir.dt.bfloat16, kind="ExternalInput"
    ).ap()
    data_gb = nc.dram_tensor(
        "data_gb", [S2, BW], mybir.dt.bfloat16, kind="ExternalInput"
    ).ap()
    data_slice = nc.dram_tensor(
        "data_slice", [P, ALL_ROWS], mybir.dt.bfloat16, kind="ExternalInput"
    ).ap()
    maskp = nc.dram_tensor(
        "maskp", [P, PACKED], mybir.dt.uint8, kind="ExternalInput"
    ).ap()
    # idx[p, 0] = poi[cell]
    idx4 = nc.dram_tensor("idx4", [P, 1], mybir.dt.int32, kind="ExternalInput").ap()
    out_t = nc.dram_tensor(
        "out_t", [P, ALL_ROWS], mybir.dt.bfloat16, kind="ExternalOutput"
    ).ap()

    with tile.TileContext(nc) as tc:
        with tc.tile_pool(name="sbuf", bufs=1) as pool:
            # mask + idx descriptors first into the (FIFO) DMA engines.
            mp = pool.tile([P, PACKED], mybir.dt.uint8, tag="mask")
            nc.scalar.dma_start(out=mp[:], in_=maskp[:])
            idx_sb = pool.tile([P, 1], mybir.dt.int32, tag="idx")
            nc.sync.dma_start(out=idx_sb[:], in_=idx4[:])

            # ---- data loads: A first (its gather waits on it) ----
            dcA = pool.tile([P, AW], mybir.dt.bfloat16, tag="dA")
            nc.sync.dma_start(out=dcA[:], in_=data_slice[:, 0:AW])
            dcB = pool.tile([P, BW], mybir.dt.bfloat16, tag="dB")
            nc.sync.dma_start(out=dcB[:], in_=data_slice[:, AW:ALL_ROWS])

            # ---- B: speculative gather, launch gated only by the idx load
            gB = pool.tile([P, BW], mybir.dt.bfloat16, tag="gB")
            nc.gpsimd.indirect_dma_start(
                out=gB[:],
                out_offset=None,
                in_=data_gb[:, :],
                in_offset=bass.IndirectOffsetOnAxis(ap=idx_sb[:, 0:1], axis=0),
                bounds_check=S2 - 1,
                oob_is_err=False,
            )

            # ---- predicate -> idx_effA on DVE (under the dc loads) ----
            idx_fA = pool.tile([P, 1], mybir.dt.float32, tag="idxf")
            nc.vector.tensor_copy(out=idx_fA[:], in_=idx_sb[:, 0:1])
            mmax = pool.tile([P, 1], mybir.dt.float32, tag="mmax")
            nc.vector.tensor_reduce(
                out=mmax[:],
                in_=mp[:],
                axis=mybir.AxisListType.X,
                op=mybir.AluOpType.max,
            )
            # shift = min(mmax, 1) * OOB: 0 for empty cells, OOB otherwise
            shift = pool.tile([P, 1], mybir.dt.float32, tag="shift")
            nc.vector.tensor_scalar(
                out=shift[:],
                in0=mmax[:],
                scalar1=1.0,
                scalar2=OOB,
                op0=mybir.AluOpType.min,
                op1=mybir.AluOpType.mult,
            )
            nc.vector.tensor_scalar(
                out=idx_fA[:],
                in0=idx_fA[:],
                scalar1=shift[:, 0:1],
                scalar2=None,
                op0=mybir.AluOpType.add,
            )
            idx_effA = pool.tile([P, 1], mybir.dt.int32, tag="idxe")
            nc.vector.tensor_copy(out=idx_effA[:], in_=idx_fA[:])
            empty = pool.tile([P, 1], mybir.dt.bfloat16, tag="empty")
            nc.vector.tensor_scalar(
                out=empty[:],
                in0=mmax[:],
                scalar1=0.0,
                scalar2=None,
                op0=mybir.AluOpType.is_equal,
            )

            # ---- A: gather-accumulate into dcA (OOB rows skipped) ----
            nc.gpsimd.indirect_dma_start(
                out=dcA[:],
                out_offset=None,
                in_=data_ga[:, :],
                in_offset=bass.IndirectOffsetOnAxis(ap=idx_effA[:], axis=0),
                bounds_check=S2 - 1,
                oob_is_err=False,
                compute_op=mybir.AluOpType.add,
            )
            nc.scalar.dma_start(out=out_t[:, 0:AW], in_=dcA[:])

            # ---- B: fused combine then store ----
            nc.vector.scalar_tensor_tensor(
                out=dcB[:],
                in0=gB[:],
                scalar=empty[:, 0:1],
                in1=dcB[:],
                op0=mybir.AluOpType.mult,
                op1=mybir.AluOpType.add,
            )
            nc.sync.dma_start(out=out_t[:, AW:ALL_ROWS], in_=dcB[:])

    nc.compile()
    return nc


def _get_program():
    if "nc" not in _CACHE:
        _CACHE["nc"] = _build_program()
    return _CACHE["nc"]


def _marshal(d, m, poi_index):
    d = np.asarray(d)
    m = np.asarray(m)
    poi_index = np.asarray(poi_index)

    # Full transposed views: [1024 cells, 4096 rows], bf16
    data_full = np.ascontiguousarray(d[:, -1].reshape(ALL_ROWS, S2).T).astype(
        ml_dtypes.bfloat16
    )
    maskp_full = np.packbits(
        m[:, -1].reshape(ALL_ROWS, S2).T != 0, axis=1
    )  # [1024, 512] u8

    poi = poi_index.astype(np.int32)

    data_ga = np.ascontiguousarray(data_full[:, :AW])
    data_gb = np.ascontiguousarray(data_full[:, AW:])

    in_maps = []
    for k in range(N_CORES):
        cells = slice(k * P, (k + 1) * P)
        idx4 = np.ascontiguousarray(poi[cells, None])  # [128, 1]
        in_maps.append(
            {
                "data_ga": data_ga,
                "data_gb": data_gb,
                "data_slice": data_full[cells],
                "maskp": maskp_full[cells],
                "idx4": idx4,
            }
        )
    return in_maps


def _unmarshal(results):
    # results[k]["out_t"] is [128 cells, 4096 rows] bf16; rows = b*64 + c.
    out = np.concatenate(
        [np.asarray(r["out_t"]) for r in results], axis=0
    )  # [1024, 4096]
    out = out.astype(np.float32).T.reshape(B, C, S2)  # [64, 64, 1024]
    return np.ascontiguousarray(out.reshape(B, C, SIDE, SIDE))


def run(d, m, poi_index, side, trace=False):
    """Run the Bass kernel; returns (output, BassKernelResults)."""
    nc = _get_program()
    in_maps = _marshal(d, m, poi_index)
    res = run_bass_kernel_spmd(
        nc, in_maps, list(range(N_CORES)), trace=trace
    )
    return _unmarshal(res.results), res


def kernel(d, m, poi_index, side):
    out, _ = run(d, m, poi_index, side)
    return out


# revision 21
# speedup vs baseline: 1.0321x; 1.0010x over previous
"""Trainium2 Bass kernel for nn_AuxCMP_61907658604772 (retrieval_knn).

Reference semantics (only the last time step of d/m matters):
    data = d[:, -1].reshape(B, C, S2)            # [64, 64, 1024] f32
    mask = m[:, -1].reshape(B, C, S2)            # [64, 64, 1024] i32 (0/1)
    cell_empty = (mask.sum(axis=(0, 1)) == 0)    # [1024] per-cell predicate
    gathered = data[:, :, poi_index]             # gather along cell dim
    out = (data + where(cell_empty, gathered, 0)).reshape(B, C, 32, 32)

Sharding: by CELLS — core k owns cells [128k, 128(k+1)) x all 4096 (b, c)
rows, in cell-major ("transposed") layout, so the empty predicate is a
core-local reduce over the cell's packed mask row and there is no
collective (an AllReduce variant measured 66us of peer-wait).

v8 design notes (measured on HW):
  * bf16 end-to-end (harness gate is rel_err < 2e-2, bf16 costs ~4e-3):
    halves every transfer vs the 29.2us f32 baseline.
  * SPECULATIVE gather: every cell pulls its poi row unconditionally; the
    two SWDGE launches are the FIRST instructions on GpSimd and wait only
    on the tiny idx load.  Gather-accumulate variants (compute_op=add,
    OOB-skip) need no DVE combine but their indirect flows run ~4x slower
    (~75-95GB/s vs ~300GB/s for bypass) AND wait on the data tile and the
    mask predicate — measured strictly worse (25.4-26.9us).
  * The per-cell select is one fused scalar_tensor_tensor per gather
    half: dc = gathered*empty + dc.  DVE is 1 elem/partition/cycle at
    0.96GHz regardless of dtype, so this costs ~2.35us per [128,2048]
    chunk — the gathers/loads overlap it only partially.
  * mask + idx descriptors are issued ahead of the 1MB of data-slice
    descriptors (DMA queues are FIFO; predicate inputs must not queue
    behind bulk data).
  * Stores are split across the two HWDGE engines (SP + Activation).

Per-core HBM traffic: 1MB slice + 1MB gather + 64KB mask + 1MB out.
"""

import numpy as np
import ml_dtypes

from concourse import bacc, bass, mybir, tile
from concourse.bass_utils import run_bass_kernel_spmd

N_CORES = 8
B, T, C, S2 = 64, 12, 64, 1024
SIDE = 32
ALL_ROWS = B * C                # 4096 (b, c) rows per cell
PACKED = ALL_ROWS // 8          # 512 packed mask bytes per cell
P = 128                         # SBUF partitions = cells per core
NG = 2                          # gather launches / combine+store chunks
GW = ALL_ROWS // NG             # rows per chunk

_CACHE = {}


def _build_program():
    nc = bacc.Bacc(
        "TRN2",
        target_bir_lowering=False,
        debug=False,
        num_devices=N_CORES,
    )
    # data_full (bf16, transposed, replicated) viewed as half-rows: cell
    # c's columns [GW*h, GW*(h+1)) live in row NG*c + h.
    data_g = nc.dram_tensor(
        "data_g", [NG * S2, GW], mybir.dt.bfloat16, kind="ExternalInput"
    ).ap()
    data_slice = nc.dram_tensor(
        "data_slice", [P, ALL_ROWS], mybir.dt.bfloat16, kind="ExternalInput"
    ).ap()
    maskp = nc.dram_tensor(
        "maskp", [P, PACKED], mybir.dt.uint8, kind="ExternalInput"
    ).ap()
    # idx[p, h] = NG*poi[cell] + h
    idx4 = nc.dram_tensor("idx4", [P, NG], mybir.dt.int32, kind="ExternalInput").ap()
    out_t = nc.dram_tensor(
        "out_t", [P, ALL_ROWS], mybir.dt.bfloat16, kind="ExternalOutput"
    ).ap()

    with tile.TileContext(nc) as tc:
        with tc.tile_pool(name="sbuf", bufs=1) as pool:
            # idx + mask descriptors first into the (FIFO) DMA engines.
            idx_sb = pool.tile([P, NG], mybir.dt.int32, tag="idx")
            nc.sync.dma_start(out=idx_sb[:], in_=idx4[:])
            mp = pool.tile([P, PACKED], mybir.dt.uint8, tag="mask")
            nc.scalar.dma_start(out=mp[:], in_=maskp[:])

            # ---- speculative gathers: first thing on GpSimd, idx-gated ----
            gts = []
            for h in range(NG):
                gt = pool.tile([P, GW], mybir.dt.bfloat16, tag=f"g{h}")
                nc.gpsimd.indirect_dma_start(
                    out=gt[:],
                    out_offset=None,
                    in_=data_g[:, :],
                    in_offset=bass.IndirectOffsetOnAxis(
                        ap=idx_sb[:, h : h + 1], axis=0
                    ),
                    bounds_check=NG * S2 - 1,
                    oob_is_err=False,
                )
                gts.append(gt)

            # ---- data loads ----
            dcs = []
            for c in range(NG):
                dc = pool.tile([P, GW], mybir.dt.bfloat16, tag=f"d{c}")
                nc.sync.dma_start(
                    out=dc[:], in_=data_slice[:, c * GW : (c + 1) * GW]
                )
                dcs.append(dc)

            # ---- per-cell empty predicate, in parallel with the gathers ----
            mmax = pool.tile([P, 1], mybir.dt.float32, tag="mmax")
            nc.vector.tensor_reduce(
                out=mmax[:],
                in_=mp[:],
                axis=mybir.AxisListType.X,
                op=mybir.AluOpType.max,
            )
            empty = pool.tile([P, 1], mybir.dt.bfloat16, tag="empty")
            nc.vector.tensor_scalar(
                out=empty[:],
                in0=mmax[:],
                scalar1=0.0,
                scalar2=None,
                op0=mybir.AluOpType.is_equal,
            )

            # ---- dc = gathered*empty + dc, then store ----
            store_eng = [nc.scalar, nc.sync]
            for c in range(NG):
                nc.vector.scalar_tensor_tensor(
                    out=dcs[c][:],
                    in0=gts[c][:],
                    scalar=empty[:, 0:1],
                    in1=dcs[c][:],
                    op0=mybir.AluOpType.mult,
                    op1=mybir.AluOpType.add,
                )
                store_eng[c % 2].dma_start(
                    out=out_t[:, c * GW : (c + 1) * GW], in_=dcs[c][:]
                )

    nc.compile()
    return nc


def _get_program():
    if "nc" not in _CACHE:
        _CACHE["nc"] = _build_program()
    return _CACHE["nc"]


def _marshal(d, m, poi_index):
    d = np.asarray(d)
    m = np.asarray(m)
    poi_index = np.asarray(poi_index)

    # Full transposed views: [1024 cells, 4096 rows], bf16
    data_full = np.ascontiguousarray(d[:, -1].reshape(ALL_ROWS, S2).T).astype(
        ml_dtypes.bfloat16
    )
    maskp_full = np.packbits(
        m[:, -1].reshape(ALL_ROWS, S2).T != 0, axis=1
    )  # [1024, 512] u8

    poi = poi_index.astype(np.int32)

    data_g = data_full.reshape(NG * S2, GW)  # view, no copy

    in_maps = []
    for k in range(N_CORES):
        cells = slice(k * P, (k + 1) * P)
        idx4 = np.ascontiguousarray(
            NG * poi[cells, None] + np.arange(NG, dtype=np.int32)[None, :]
        )  # [128, NG]
        in_maps.append(
            {
                "data_g": data_g,
                "data_slice": data_full[cells],
                "maskp": maskp_full[cells],
                "idx4": idx4,
            }
        )
    return in_maps


def _unmarshal(results):
    # results[k]["out_t"] is [128 cells, 4096 rows] bf16; rows = b*64 + c.
    out = np.concatenate(
        [np.asarray(r["out_t"]) for r in results], axis=0
    )  # [1024, 4096]
    out = out.astype(np.float32).T.reshape(B, C, S2)  # [64, 64, 1024]
    return np.ascontiguousarray(out.reshape(B, C, SIDE, SIDE))


def run(d, m, poi_index, side, trace=False):
    """Run the Bass kernel; returns (output, BassKernelResults)."""
    nc = _get_program()
    in_maps = _marshal(d, m, poi_index)
    res = run_bass_kernel_spmd(
        nc, in_maps, list(range(N_CORES)), trace=trace
    )
    return _unmarshal(res.results), res


def kernel(d, m, poi_index, side):
    out, _ = run(d, m, poi_index, side)
    return out
